# revision 1
# baseline (speedup 1.0000x reference)
"""AttentionRPE kernel for 8 Trainium2 NeuronCores.

Math (per (b,s) row, T=128 targets, D=256, H=8 heads, DH=32, DR=32):
  q   = src @ Wsrc.T + bsrc                       [D]
  K'  = tgt @ Wk.T + rpe @ Rwk.T (+const bias)    [T, D]
  V'  = tgt @ Wv.T + rpe @ Rwv.T (+const bias)    [T, D]
  att = softmax_h(q_h . K'_h / sqrt(DH))          [H, T]   (masked)
  out = (att @ V')_heads @ Wout.T + bout          [D]

Device formulation (the tricks that make it fast):
  * K-path q-fold: logits[h,t] = sum_d qw[h,d]*tgtx[t,d], with
    qw = (q/sqrt(DH)) @ Wkx  folded per row (tiny), tgtx = [tgt | rpe]
    ([T, 288]).  Only tgtx must be transposed on-chip (PE transposes).
  * V-path commute: G[d,h] = sum_t tgtx[t,d]*att[h,t] first (natural
    layout matmul), then out = sum_{h,d} G[d,h]*Wfx[h,d,:] where
    Wfx[h] = (Wout[:,hslice] @ Wvx[hslice,:]).T is precomputed on host.
    The V/rpe_v projection of the big tensor is never materialized.
  * Padding mask + off-diagonal garbage masking folded into one extra
    accumulating matmul into the logits PSUM (rank-16 selector A16 @ Bm).
  * All biases except bsrc are constant in t and either cancel in
    softmax (K-side) or fold into the output bias (V-side).

Sharding: 1024 (b,s) rows split contiguously over 8 cores (128 each).
"""

import numpy as np

import concourse.bass as bass
import concourse.bacc as bacc
import concourse.mybir as mybir
from concourse.tile import TileContext
from concourse.masks import make_identity
from concourse.bass_utils import run_bass_kernel_spmd

B, S, T, D = 2, 512, 128, 256
H, DH, DR = 8, 32, 32
DX = D + DR          # 288 = tgt|rpe feature dim
DOUT = D
NCORES = 8
BS = B * S           # 1024 total rows
SC = BS // NCORES    # 128 rows per core

F32 = mybir.dt.float32
F32R = mybir.dt.float32r

AX = mybir.AxisListType
ALU = mybir.AluOpType
ACTF = mybir.ActivationFunctionType

# float32r (full-rate, reduced-precision fp32) per matmul group.
R_MASK = True      # mask add matmul (values 0/-1e30: always safe)
R_TRANSP = False  # (transposes stay exact fp32)
R_LOGITS = True   # (via float32r-typed operand tiles)
R_GT = False       # G = att @ tgtx matmuls
R_FINAL = True    # output projection matmuls
R_QPATH = False    # q / qw / qrw setup matmuls


def _r(ap, on):
    return ap.bitcast(F32R) if on else ap


def build(sc=SC):
    """Build the per-core Bass program. sc = rows per core (multiple of 16)."""
    assert sc % 16 == 0
    nblk = sc // 16
    nc = bacc.Bacc()

    src_d = nc.dram_tensor("src", [sc, D], F32, kind="ExternalInput")
    tgtx_d = nc.dram_tensor("tgtx", [sc, T, DX], F32, kind="ExternalInput")
    bm_d = nc.dram_tensor("bm", [nblk, 16, 512], F32, kind="ExternalInput")
    a16x_d = nc.dram_tensor("a16x", [16, sc, H], F32, kind="ExternalInput")
    wsrcT_d = nc.dram_tensor("wsrcT", [D, D], F32, kind="ExternalInput")
    wk_d = nc.dram_tensor("wk", [D, D], F32, kind="ExternalInput")
    rwk_d = nc.dram_tensor("rwk", [D, DR], F32, kind="ExternalInput")
    wfx_d = nc.dram_tensor("wfx", [H, DX, DOUT], F32R, kind="ExternalInput")
    bsrc_d = nc.dram_tensor("bsrc", [D, 1], F32, kind="ExternalInput")
    obias_d = nc.dram_tensor("obias", [sc, DOUT], F32, kind="ExternalInput")
    rmask_d = nc.dram_tensor("rmask", [sc, 1], F32, kind="ExternalInput")
    out_d = nc.dram_tensor("out", [sc, DOUT], F32, kind="ExternalOutput")

    with TileContext(nc) as tc:
        with (
            tc.tile_pool(name="const", bufs=1) as cp,
            tc.tile_pool(name="txp", bufs=10) as txp,
            tc.tile_pool(name="txtp", bufs=3) as txtp,
            tc.tile_pool(name="attnp", bufs=2) as attnp,
            tc.tile_pool(name="smallp", bufs=2) as smallp,
            tc.tile_pool(name="ps_tr", bufs=1, space="PSUM") as ps_tr,
            tc.tile_pool(name="ps_tr2", bufs=1, space="PSUM") as ps_tr2,
            tc.tile_pool(name="ps_l", bufs=2, space="PSUM") as ps_l,
            tc.tile_pool(name="ps_misc", bufs=1, space="PSUM") as ps_misc,
        ):
            # ---------------- constants & weights ----------------
            eye = cp.tile([128, 128], F32, name="eye")
            make_identity(nc, eye)
            wsrcT = []
            wk = []
            rwk = []
            for c in range(2):
                w1 = cp.tile([128, D], F32, name=f"wsrcT{c}")
                nc.sync.dma_start(out=w1, in_=wsrcT_d[c * 128:(c + 1) * 128, :])
                wsrcT.append(w1)
                w2 = cp.tile([128, D], F32, name=f"wk{c}")
                nc.sync.dma_start(out=w2, in_=wk_d[c * 128:(c + 1) * 128, :])
                wk.append(w2)
                w3 = cp.tile([128, DR], F32, name=f"rwk{c}")
                nc.sync.dma_start(out=w3, in_=rwk_d[c * 128:(c + 1) * 128, :])
                rwk.append(w3)
            wfx_main = {}
            wfx_r = {}
            for h in range(H):
                for c in range(2):
                    wt = cp.tile([128, DOUT], F32R, name=f"wfx{h}_{c}")
                    nc.sync.dma_start(
                        out=wt, in_=wfx_d[h, c * 128:(c + 1) * 128, :])
                    wfx_main[(h, c)] = wt
                wt = cp.tile([32, DOUT], F32R, name=f"wfxr{h}")
                nc.sync.dma_start(out=wt, in_=wfx_d[h, D:DX, :])
                wfx_r[h] = wt
            bsrc = []
            for c in range(2):
                bt = cp.tile([128, 1], F32, name=f"bsrc{c}")
                nc.sync.dma_start(out=bt, in_=bsrc_d[c * 128:(c + 1) * 128, :])
                bsrc.append(bt)
            obias = cp.tile([sc, DOUT], F32, name="obias")
            nc.sync.dma_start(out=obias, in_=obias_d[:, :])
            rmask = cp.tile([sc, 1], F32, name="rmask")
            nc.sync.dma_start(out=rmask, in_=rmask_d[:, :])

            gall = cp.tile([128, sc, 24], F32R, name="gall")

            # ---------------- q path (once per core) ----------------
            src_sb = cp.tile([sc, D], F32, name="src_sb")
            nc.sync.dma_start(out=src_sb, in_=src_d[:, :])
            srcT = []
            for c in range(2):
                st_ps = ps_misc.tile([128, sc], F32, tag="mA", name="st_ps", bufs=2)
                nc.tensor.transpose(st_ps, src_sb[:, c * 128:(c + 1) * 128],
                                    eye[0:sc, 0:sc])
                st = cp.tile([128, sc], F32, name=f"srcT{c}")
                nc.vector.tensor_copy(st, st_ps)
                srcT.append(st)
            qT = []
            for ec in range(2):
                q_ps = ps_misc.tile([128, sc], F32, tag="mB", name="q_ps")
                for dc in range(2):
                    nc.tensor.matmul(
                        q_ps,
                        _r(wsrcT[dc][:, ec * 128:(ec + 1) * 128], R_QPATH),
                        _r(srcT[dc], R_QPATH),
                        start=(dc == 0), stop=(dc == 1))
                qt = cp.tile([128, sc], F32, name=f"qT{ec}")
                nc.vector.tensor_scalar_add(qt, q_ps, bsrc[ec])
                qT.append(qt)
            qwT = []
            for dc in range(2):
                qwT.append(cp.tile([128, sc, H], F32, name=f"qwT{dc}"))
            qwTf = [t.rearrange("p s h -> p (s h)") for t in qwT]
            qrwT = cp.tile([48, sc, H], F32, name="qrwT")
            qrwTf = qrwT.rearrange("p s h -> p (s h)")
            nc.sync.dma_start(out=qrwT[32:48, :, :], in_=a16x_d[:, :, :])
            for h in range(H):
                ti, ro = h // 4, (h % 4) * 32
                for dc in range(2):
                    qw_ps = ps_misc.tile([128, sc], F32, tag="mA", name="qw_ps", bufs=2)
                    nc.tensor.matmul(
                        qw_ps,
                        _r(wk[ti][ro:ro + 32, dc * 128:(dc + 1) * 128], R_QPATH),
                        _r(qT[ti][ro:ro + 32, :], R_QPATH),
                        start=True, stop=True, tile_position=(ro, 0))
                    if dc == 0:
                        nc.vector.tensor_copy(qwT[dc][:, :, h], qw_ps)
                    else:
                        nc.scalar.activation(qwT[dc][:, :, h], qw_ps, ACTF.Copy)
                qr_ps = ps_misc.tile([32, sc], F32, tag="mB", name="qr_ps")
                nc.tensor.matmul(
                    qr_ps,
                    _r(rwk[ti][ro:ro + 32, :], R_QPATH),
                    _r(qT[ti][ro:ro + 32, :], R_QPATH),
                    start=True, stop=True, tile_position=(ro, 0))
                nc.vector.tensor_copy(qrwT[0:32, :, h], qr_ps)

            # ---------------- main loop ----------------
            for blk in range(nblk):
                l_ps = ps_l.tile([128, 512], F32, name="l_ps")
                tx_tiles = []
                for g in range(4):
                    c0 = ps_tr.tile([128, 512], F32, tag="c0", name="c0")
                    c1 = ps_tr.tile([128, 512], F32, tag="c1", name="c1")
                    c2 = ps_tr2.tile([32, 512], F32, tag="c2", name="c2")
                    s0 = blk * 16 + g * 4
                    tx4 = txp.tile([T, 4, DX], F32, tag="tx", name="tx4")
                    nc.sync.dma_start(
                        out=tx4, in_=tgtx_d[s0:s0 + 4, :, :].transpose([1, 0, 2]))
                    tx_tiles.append(tx4)
                    for sg in range(4):
                        tx = tx4[:, sg, :]
                        sl = slice(sg * 128, (sg + 1) * 128)
                        nc.tensor.matmul(
                            c0[:, sl], tx[:, 0:128],
                            eye,
                            start=True, stop=True, is_transpose=True)
                        nc.tensor.matmul(
                            c1[:, sl], tx[:, 128:256],
                            eye,
                            start=True, stop=True, is_transpose=True)
                        nc.tensor.matmul(
                            c2[:, sl], tx[:, 256:288],
                            eye,
                            start=True, stop=True, is_transpose=True)
                    t0 = txtp.tile([128, 512], F32, tag="t0", name="t0")
                    t1 = txtp.tile([128, 512], F32, tag="t1", name="t1")
                    t2 = txtp.tile([48, 512], F32, tag="t2", name="t2")
                    nc.sync.dma_start(out=t2[32:48, :], in_=bm_d[blk, :, :])
                    nc.vector.tensor_copy(t0, c0)
                    nc.scalar.activation(t1, c1, ACTF.Copy)
                    nc.vector.tensor_copy(t2[0:32, :], c2)
                    gs = blk * 16 + g * 4
                    osl = slice(g * 32, (g + 1) * 32)
                    nc.tensor.matmul(
                        l_ps[osl, :],
                        qwTf[0][:, gs * 8:gs * 8 + 32],
                        t0, start=True, stop=False,
                        tile_position=(0, g * 32))
                    nc.tensor.matmul(
                        l_ps[osl, :],
                        qwTf[1][:, gs * 8:gs * 8 + 32],
                        t1, start=False, stop=False,
                        tile_position=(0, g * 32))
                    nc.tensor.matmul(
                        l_ps[osl, :],
                        qrwTf[:, gs * 8:gs * 8 + 32],
                        t2, start=False, stop=True,
                        tile_position=(0, g * 32))

                # softmax over the 512-wide rows (off-diag blocks masked to 0)
                nmx = smallp.tile([128, 1], F32, tag="nmx", name="nmx")
                nc.vector.tensor_reduce(nmx, l_ps, axis=AX.X, op=ALU.max,
                                        negate=True)
                den = smallp.tile([128, 1], F32, tag="den", name="den")
                attn_e = attnp.tile([128, 512], F32, tag="ae", name="attn_e")
                nc.scalar.activation(attn_e, l_ps, ACTF.Exp, bias=nmx,
                                     scale=1.0, accum_out=den)
                rden = smallp.tile([128, 1], F32, tag="rden", name="rden")
                nc.vector.reciprocal(rden, den)
                attn_n = attnp.tile([128, 512], F32, tag="an", name="attn_n")
                nc.vector.tensor_scalar_mul(attn_n, attn_e, rden)

                # Off-slot attn entries are exactly 0 (mask -1e30 -> exp -> 0),
                # so the sum of the 4 block transposes is the exact attnT.
                at_ps = ps_misc.tile([128, 128], F32, tag="mA", name="at_ps", bufs=2)
                for g in range(4):
                    nc.tensor.matmul(
                        at_ps,
                        attn_n[:, g * 128:(g + 1) * 128],
                        eye,
                        start=(g == 0), stop=(g == 3), is_transpose=True)
                atT = smallp.tile([128, 128], F32, tag="atT", name="atT")
                nc.vector.tensor_copy(atT, at_ps)

                gt_ps = ps_misc.tile([128, 16, 24], F32, tag="mB", name="gt_ps")
                for j in range(16):
                    tx = tx_tiles[j // 4][:, j % 4, :]
                    av = atT[:, j * 8:(j + 1) * 8]
                    nc.tensor.matmul(gt_ps[:, j, 0:8],
                                     _r(tx[:, 0:128], R_GT), _r(av, R_GT),
                                     start=True, stop=True)
                    nc.tensor.matmul(gt_ps[:, j, 8:16],
                                     _r(tx[:, 128:256], R_GT), _r(av, R_GT),
                                     start=True, stop=True)
                    nc.tensor.matmul(gt_ps[0:32, j, 16:24],
                                     _r(tx[:, 256:288], R_GT), _r(av, R_GT),
                                     start=True, stop=True)
                bsl = slice(blk * 16, (blk + 1) * 16)
                nc.vector.tensor_copy(gall[:, bsl, 0:16], gt_ps[:, :, 0:16])
                nc.scalar.activation(gall[0:32, bsl, 16:24],
                                     gt_ps[0:32, :, 16:24], ACTF.Copy)

            # ---------------- output projection ----------------
            out_ps = ps_misc.tile([sc, DOUT], F32, tag="mA", name="out_ps", bufs=2)
            for h in range(H):
                for c in range(2):
                    nc.tensor.matmul(
                        out_ps,
                        gall[:, :, c * 8 + h],
                        wfx_main[(h, c)],
                        start=(h == 0 and c == 0), stop=False)
                nc.tensor.matmul(
                    out_ps,
                    gall[0:32, :, 16 + h],
                    wfx_r[h],
                    start=False, stop=(h == H - 1))
            out_sb = cp.tile([sc, DOUT], F32, name="out_sb")
            nc.vector.tensor_tensor(out_sb, out_ps, obias, op=ALU.add)
            out_sb2 = cp.tile([sc, DOUT], F32, name="out_sb2")
            nc.vector.tensor_scalar_mul(out_sb2, out_sb, rmask)
            nc.sync.dma_start(out=out_d[:, :], in_=out_sb2)

    nc.finalize()
    return nc


def host_prep(src, tgt, rpe, tgt_padding_mask, in_proj_weight, in_proj_bias,
              out_proj_weight, out_proj_bias, rpe_weight, rpe_bias):
    """Host-side slicing/weight prep. Returns per-core input maps."""
    f = np.float32
    scale = f(1.0 / np.sqrt(DH))
    src_f = np.ascontiguousarray(np.asarray(src, f).reshape(BS, D))
    tgtx = np.concatenate(
        [np.asarray(tgt, f).reshape(BS, T, D),
         np.asarray(rpe, f).reshape(BS, T, DR)], axis=-1)
    mask = np.asarray(tgt_padding_mask, bool).reshape(BS, T)
    no_valid = mask.all(-1)
    maskadd = np.where(mask & ~no_valid[:, None], f(-1e30), f(0.0)).astype(f)
    rowmask = np.ascontiguousarray((~no_valid).astype(f)[:, None])

    nblk_total = BS // 16
    bm = np.full((nblk_total, 16, 4, T), -1e30, f)
    ma_b = maskadd.reshape(nblk_total, 16, T)
    for j in range(16):
        bm[:, j, j % 4, :] = ma_b[:, j, :]
    bm = bm.reshape(nblk_total, 16, 512)
    sidx = np.arange(SC) % 16
    a16x = (np.arange(16)[:, None, None] == sidx[None, :, None]).astype(f)
    a16x = np.ascontiguousarray(np.broadcast_to(a16x, (16, SC, H)))

    ipw = np.asarray(in_proj_weight, f)
    ipb = np.asarray(in_proj_bias, f)
    opw = np.asarray(out_proj_weight, f)
    opb = np.asarray(out_proj_bias, f)
    rw = np.asarray(rpe_weight, f)
    rb = np.asarray(rpe_bias, f)

    wsrcT = np.ascontiguousarray(ipw[:D].T * scale)          # [d, e]
    bsrc = np.ascontiguousarray((ipb[:D] * scale)[:, None])  # [D, 1]
    wk = np.ascontiguousarray(ipw[D:2 * D])                  # [e, d]
    rwk = np.ascontiguousarray(rw[:D])                       # [e, r]
    wvx = np.concatenate([ipw[2 * D:3 * D], rw[D:2 * D]], axis=1)  # [e, 288]
    wfx = np.empty((H, DX, DOUT), f)
    for h in range(H):
        hs = slice(h * 32, (h + 1) * 32)
        wfx[h] = (opw[:, hs] @ wvx[hs, :]).T
    obias = (opb + opw @ (ipb[2 * D:3 * D] + rb[D:2 * D]))[None, :]
    obias = np.ascontiguousarray(np.repeat(obias.astype(f), SC, axis=0))

    wfx = round_f32r(wfx)

    nblk = SC // 16
    in_maps = []
    for c in range(NCORES):
        sl = slice(c * SC, (c + 1) * SC)
        in_maps.append({
            "src": src_f[sl],
            "tgtx": np.ascontiguousarray(tgtx[sl]),
            "bm": np.ascontiguousarray(bm[c * nblk:(c + 1) * nblk]),
            "a16x": a16x,
            "wsrcT": wsrcT,
            "wk": wk,
            "rwk": rwk,
            "wfx": wfx,
            "bsrc": bsrc,
            "obias": obias,
            "rmask": rowmask[sl],
        })
    return in_maps


def round_f32r(x):
    """Round fp32 array to the fp32r grid (RNE to 11 mantissa bits)."""
    u = np.ascontiguousarray(x, np.float32).view(np.uint32)
    u = (u + 0x7FF + ((u >> 12) & 1)) & 0xFFFFF000
    return u.astype(np.uint32).view(np.float32)


_NC_CACHE = {}


def get_nc(sc=SC):
    if sc not in _NC_CACHE:
        _NC_CACHE[sc] = build(sc)
    return _NC_CACHE[sc]


def run(in_maps, trace=False):
    nc = get_nc(SC)
    return run_bass_kernel_spmd(nc, in_maps, list(range(NCORES)), trace=trace)


def kernel(**inputs):
    in_maps = host_prep(**inputs)
    res = run(in_maps).results
    out = np.concatenate([res[c]["out"] for c in range(NCORES)], axis=0)
    return np.ascontiguousarray(out.reshape(B, S, D))



# revision 2
# speedup vs baseline: 2.3841x; 2.3841x over previous
"""AttentionRPE kernel for 8 Trainium2 NeuronCores.

Math (per (b,s) row, T=128 targets, D=256, H=8 heads, DH=32, DR=32):
  q   = src @ Wsrc.T + bsrc                       [D]
  K'  = tgt @ Wk.T + rpe @ Rwk.T (+const bias)    [T, D]
  V'  = tgt @ Wv.T + rpe @ Rwv.T (+const bias)    [T, D]
  att = softmax_h(q_h . K'_h / sqrt(DH))          [H, T]   (masked)
  out = (att @ V')_heads @ Wout.T + bout          [D]

Device formulation:
  * K-path q-fold: logits[h,t] = sum_d qw[h,d]*tgtx[t,d], with
    qw = (q/sqrt(DH)) @ Wkx  folded per row (tiny), tgtx = [tgt | rpe].
  * The big tensor is shipped in BOTH layouts, fp16, prepared on host:
    natural [t, d] for the V-path and transposed [d, (s,t)] for the
    K-path.  Same total bytes as one fp32 copy; zero on-chip transposes
    of tgtx.
  * V-path commute: G[d,h] = sum_t tgtx[t,d]*att[h,t] first (natural
    layout matmul), then out = sum_{h,d} G[d,h]*Wfx[h,d,:] where
    Wfx[h] = (Wout[:,hslice] @ Wvx[hslice,:]).T is precomputed on host.
  * Padding mask + off-diagonal garbage masking folded into one extra
    accumulating matmul into the logits PSUM (rank-16 selector A16 @ Bm);
    the Bm rows ride along in the transposed-layout DMA.
  * Attention transpose: 4 fp16 PE transposes per 16-row block into
    disjoint PSUM chunks; off-diagonal attn entries are exactly 0, so
    each row's G matmul just reads its own chunk (no summation).

Sharding: 1024 (b,s) rows split contiguously over 8 cores (128 each).
"""

import numpy as np

import concourse.bass as bass
import concourse.bacc as bacc
import concourse.mybir as mybir
from concourse.tile import TileContext
from concourse.masks import make_identity
from concourse.bass_utils import run_bass_kernel_spmd

B, S, T, D = 2, 512, 128, 256
H, DH, DR = 8, 32, 32
DX = D + DR          # 288 = tgt|rpe feature dim
DXM = DX + 16        # 304 = transposed layout rows (288 features + 16 bm rows)
DOUT = D
NCORES = 8
BS = B * S           # 1024 total rows
SC = BS // NCORES    # 128 rows per core
MASKV = -60000.0     # fits fp16; exp() still underflows to exactly 0

F32 = mybir.dt.float32
F32R = mybir.dt.float32r
F16 = mybir.dt.float16

AX = mybir.AxisListType
ALU = mybir.AluOpType
ACTF = mybir.ActivationFunctionType


def build(sc=SC):
    """Build the per-core Bass program. sc = rows per core (multiple of 16)."""
    assert sc % 16 == 0
    nblk = sc // 16
    ngr = sc // 4
    nc = bacc.Bacc()

    src_d = nc.dram_tensor("src", [sc, D], F32, kind="ExternalInput")
    txn_d = nc.dram_tensor("txn", [ngr, T, 4 * DX], F16, kind="ExternalInput")
    txt_d = nc.dram_tensor("txt", [ngr, DXM, 512], F16, kind="ExternalInput")
    a16x_d = nc.dram_tensor("a16x", [16, sc, H], F16, kind="ExternalInput")
    wsrcT_d = nc.dram_tensor("wsrcT", [D, D], F32, kind="ExternalInput")
    wk_d = nc.dram_tensor("wk", [D, D], F32, kind="ExternalInput")
    rwk_d = nc.dram_tensor("rwk", [D, DR], F32, kind="ExternalInput")
    wfx_d = nc.dram_tensor("wfx", [H, DX, DOUT], F32R, kind="ExternalInput")
    bsrc_d = nc.dram_tensor("bsrc", [D, 1], F32, kind="ExternalInput")
    obias_d = nc.dram_tensor("obias", [sc, DOUT], F32, kind="ExternalInput")
    rmask_d = nc.dram_tensor("rmask", [sc, 1], F32, kind="ExternalInput")
    out_d = nc.dram_tensor("out", [sc, DOUT], F32, kind="ExternalOutput")

    with TileContext(nc) as tc:
        with (
            tc.tile_pool(name="const", bufs=1) as cp,
            tc.tile_pool(name="txp", bufs=6) as txp,
            tc.tile_pool(name="txtp", bufs=6) as txtp,
            tc.tile_pool(name="attnp", bufs=2) as attnp,
            tc.tile_pool(name="smallp", bufs=2) as smallp,
            tc.tile_pool(name="ps_l", bufs=2, space="PSUM") as ps_l,
            tc.tile_pool(name="ps_at", bufs=1, space="PSUM") as ps_at,
            tc.tile_pool(name="ps_g", bufs=2, space="PSUM") as ps_g,
            tc.tile_pool(name="ps_misc", bufs=1, space="PSUM") as ps_misc,
        ):
            # ---------------- constants & early weights ----------------
            eye = cp.tile([128, 128], F32, name="eye")
            make_identity(nc, eye)
            eye16 = cp.tile([128, 128], F16, name="eye16")
            make_identity(nc, eye16)
            wsrcT = []
            wk = []
            rwk = []
            for c in range(2):
                w1 = cp.tile([128, D], F32, name=f"wsrcT{c}")
                nc.sync.dma_start(out=w1, in_=wsrcT_d[c * 128:(c + 1) * 128, :])
                wsrcT.append(w1)
                w2 = cp.tile([128, D], F32, name=f"wk{c}")
                nc.sync.dma_start(out=w2, in_=wk_d[c * 128:(c + 1) * 128, :])
                wk.append(w2)
                w3 = cp.tile([128, DR], F32, name=f"rwk{c}")
                nc.sync.dma_start(out=w3, in_=rwk_d[c * 128:(c + 1) * 128, :])
                rwk.append(w3)
            bsrc = []
            for c in range(2):
                bt = cp.tile([128, 1], F32, name=f"bsrc{c}")
                nc.sync.dma_start(out=bt, in_=bsrc_d[c * 128:(c + 1) * 128, :])
                bsrc.append(bt)

            gall = cp.tile([128, sc, 24], F32R, name="gall")

            # ---------------- q path (once per core) ----------------
            src_sb = cp.tile([sc, D], F32, name="src_sb")
            nc.sync.dma_start(out=src_sb, in_=src_d[:, :])
            srcT = []
            for c in range(2):
                st_ps = ps_misc.tile([128, sc], F32, tag="mA", name="st_ps", bufs=2)
                nc.tensor.transpose(st_ps, src_sb[:, c * 128:(c + 1) * 128],
                                    eye[0:sc, 0:sc])
                st = cp.tile([128, sc], F32, name=f"srcT{c}")
                nc.vector.tensor_copy(st, st_ps)
                srcT.append(st)
            qT = []
            for ec in range(2):
                q_ps = ps_misc.tile([128, sc], F32, tag="mB", name="q_ps")
                for dc in range(2):
                    nc.tensor.matmul(
                        q_ps,
                        wsrcT[dc][:, ec * 128:(ec + 1) * 128],
                        srcT[dc],
                        start=(dc == 0), stop=(dc == 1))
                qt = cp.tile([128, sc], F32, name=f"qT{ec}")
                nc.vector.tensor_scalar_add(qt, q_ps, bsrc[ec])
                qT.append(qt)
            qwT = []
            for dc in range(2):
                qwT.append(cp.tile([128, sc, H], F16, name=f"qwT{dc}"))
            qwTf = [t.rearrange("p s h -> p (s h)") for t in qwT]
            qrwT = cp.tile([48, sc, H], F16, name="qrwT")
            qrwTf = qrwT.rearrange("p s h -> p (s h)")
            nc.sync.dma_start(out=qrwT[32:48, :, :], in_=a16x_d[:, :, :])
            for h in range(H):
                ti, ro = h // 4, (h % 4) * 32
                for dc in range(2):
                    qw_ps = ps_misc.tile([128, sc], F32, tag="mA", name="qw_ps", bufs=2)
                    nc.tensor.matmul(
                        qw_ps,
                        wk[ti][ro:ro + 32, dc * 128:(dc + 1) * 128],
                        qT[ti][ro:ro + 32, :],
                        start=True, stop=True, tile_position=(ro, 0))
                    if dc == 0:
                        nc.vector.tensor_copy(qwT[dc][:, :, h], qw_ps)
                    else:
                        nc.scalar.activation(qwT[dc][:, :, h], qw_ps, ACTF.Copy)
                qr_ps = ps_misc.tile([32, sc], F32, tag="mB", name="qr_ps")
                nc.tensor.matmul(
                    qr_ps,
                    rwk[ti][ro:ro + 32, :],
                    qT[ti][ro:ro + 32, :],
                    start=True, stop=True, tile_position=(ro, 0))
                nc.vector.tensor_copy(qrwT[0:32, :, h], qr_ps)

            # late-needed weights: issue during the main loop (DMA has slack)
            wfx_main = {}
            wfx_r = {}
            obias = cp.tile([sc, DOUT], F32, name="obias")
            rmask = cp.tile([sc, 1], F32, name="rmask")

            def issue_wfx(h):
                for c in range(2):
                    wt = cp.tile([128, DOUT], F32R, name=f"wfx{h}_{c}")
                    nc.sync.dma_start(
                        out=wt, in_=wfx_d[h, c * 128:(c + 1) * 128, :])
                    wfx_main[(h, c)] = wt
                wt = cp.tile([32, DOUT], F32R, name=f"wfxr{h}")
                nc.sync.dma_start(out=wt, in_=wfx_d[h, D:DX, :])
                wfx_r[h] = wt

            # ---------------- main loop ----------------
            for blk in range(nblk):
                l_ps = ps_l.tile([128, 512], F32, name="l_ps")
                tx_tiles = []
                for g in range(4):
                    gi = blk * 4 + g
                    t0 = txtp.tile([128, 512], F16, tag="t0", name="t0")
                    nc.sync.dma_start(out=t0, in_=txt_d[gi, 0:128, :])
                    t1 = txtp.tile([128, 512], F16, tag="t1", name="t1")
                    nc.sync.dma_start(out=t1, in_=txt_d[gi, 128:256, :])
                    t2 = txtp.tile([48, 512], F16, tag="t2", name="t2")
                    nc.sync.dma_start(out=t2, in_=txt_d[gi, 256:304, :])
                    tx4 = txp.tile([T, 4, DX], F16, tag="tx", name="tx4")
                    nc.sync.dma_start(
                        out=tx4.rearrange("t f d -> t (f d)"), in_=txn_d[gi, :, :])
                    tx_tiles.append(tx4)
                    gs = gi
                    osl = slice(g * 32, (g + 1) * 32)
                    nc.tensor.matmul(
                        l_ps[osl, :],
                        qwTf[0][:, gs * 32:gs * 32 + 32],
                        t0, start=True, stop=False,
                        tile_position=(0, g * 32))
                    nc.tensor.matmul(
                        l_ps[osl, :],
                        qwTf[1][:, gs * 32:gs * 32 + 32],
                        t1, start=False, stop=False,
                        tile_position=(0, g * 32))
                    nc.tensor.matmul(
                        l_ps[osl, :],
                        qrwTf[:, gs * 32:gs * 32 + 32],
                        t2, start=False, stop=True,
                        tile_position=(0, g * 32))

                if blk < H:
                    issue_wfx(blk)
                if blk == 4:
                    nc.sync.dma_start(out=obias, in_=obias_d[:, :])
                    nc.sync.dma_start(out=rmask, in_=rmask_d[:, :])

                # softmax over the 512-wide rows (off-diag blocks masked)
                nmx = smallp.tile([128, 1], F32, tag="nmx", name="nmx")
                nc.vector.tensor_reduce(nmx, l_ps, axis=AX.X, op=ALU.max,
                                        negate=True)
                den = smallp.tile([128, 1], F32, tag="den", name="den")
                attn_e = attnp.tile([128, 512], F16, tag="ae", name="attn_e")
                nc.scalar.activation(attn_e, l_ps, ACTF.Exp, bias=nmx,
                                     scale=1.0, accum_out=den)
                rden = smallp.tile([128, 1], F32, tag="rden", name="rden")
                nc.vector.reciprocal(rden, den)
                attn_n = attnp.tile([128, 512], F16, tag="an", name="attn_n")
                nc.vector.tensor_scalar_mul(attn_n, attn_e, rden)

                # Off-slot attn entries are exactly 0; each row's G reads its
                # own transposed chunk, so no cross-chunk sum is needed.
                at4_ps = ps_at.tile([128, 4, 128], F16, tag="at4", name="at4_ps")
                for g in range(4):
                    nc.tensor.matmul(
                        at4_ps[:, g, :],
                        attn_n[:, g * 128:(g + 1) * 128],
                        eye16,
                        start=True, stop=True, is_transpose=True)
                atT4 = smallp.tile([128, 4, 128], F16, tag="atT", name="atT4")
                nc.vector.tensor_copy(atT4, at4_ps)

                gt_ps = ps_g.tile([128, 16, 24], F32, tag="gt", name="gt_ps")
                for j in range(16):
                    tx = tx_tiles[j // 4][:, j % 4, :]
                    av = atT4[:, j % 4, j * 8:(j + 1) * 8]
                    nc.tensor.matmul(gt_ps[:, j, 0:8],
                                     tx[:, 0:128], av,
                                     start=True, stop=True)
                    nc.tensor.matmul(gt_ps[:, j, 8:16],
                                     tx[:, 128:256], av,
                                     start=True, stop=True)
                    nc.tensor.matmul(gt_ps[0:32, j, 16:24],
                                     tx[:, 256:288], av,
                                     start=True, stop=True)
                bsl = slice(blk * 16, (blk + 1) * 16)
                nc.vector.tensor_copy(gall[:, bsl, 0:16], gt_ps[:, :, 0:16])
                nc.scalar.activation(gall[0:32, bsl, 16:24],
                                     gt_ps[0:32, :, 16:24], ACTF.Copy)

            # ---------------- output projection ----------------
            out_ps = ps_misc.tile([sc, DOUT], F32, tag="mA", name="out_ps", bufs=2)
            for h in range(H):
                for c in range(2):
                    nc.tensor.matmul(
                        out_ps,
                        gall[:, :, c * 8 + h],
                        wfx_main[(h, c)],
                        start=(h == 0 and c == 0), stop=False)
                nc.tensor.matmul(
                    out_ps,
                    gall[0:32, :, 16 + h],
                    wfx_r[h],
                    start=False, stop=(h == H - 1))
            out_sb = cp.tile([sc, DOUT], F32, name="out_sb")
            nc.vector.tensor_tensor(out_sb, out_ps, obias, op=ALU.add)
            out_sb2 = cp.tile([sc, DOUT], F32, name="out_sb2")
            nc.vector.tensor_scalar_mul(out_sb2, out_sb, rmask)
            nc.sync.dma_start(out=out_d[:, :], in_=out_sb2)

    nc.finalize()
    return nc


def host_prep(src, tgt, rpe, tgt_padding_mask, in_proj_weight, in_proj_bias,
              out_proj_weight, out_proj_bias, rpe_weight, rpe_bias):
    """Host-side slicing/weight prep. Returns per-core input maps."""
    f = np.float32
    f16 = np.float16
    scale = f(1.0 / np.sqrt(DH))
    src_f = np.ascontiguousarray(np.asarray(src, f).reshape(BS, D))
    tgtx = np.concatenate(
        [np.asarray(tgt, f).reshape(BS, T, D),
         np.asarray(rpe, f).reshape(BS, T, DR)], axis=-1).astype(f16)
    mask = np.asarray(tgt_padding_mask, bool).reshape(BS, T)
    no_valid = mask.all(-1)
    maskadd = np.where(mask & ~no_valid[:, None], f16(MASKV), f16(0.0))
    rowmask = np.ascontiguousarray((~no_valid).astype(f)[:, None])

    # bm[blk, j, :] : row j's mask at its own group slot, MASKV elsewhere
    nblk_total = BS // 16
    bm = np.full((nblk_total, 16, 4, T), f16(MASKV), f16)
    ma_b = maskadd.reshape(nblk_total, 16, T)
    for j in range(16):
        bm[:, j, j % 4, :] = ma_b[:, j, :]
    bm = bm.reshape(nblk_total, 16, 512)

    # natural layout, group-packed: [ngr, T, 4*DX]
    ngr_total = BS // 4
    txn = np.ascontiguousarray(
        tgtx.reshape(ngr_total, 4, T, DX).transpose(0, 2, 1, 3)
    ).reshape(ngr_total, T, 4 * DX)
    # transposed layout + bm rows: [ngr, DXM, 512]
    txt = np.empty((ngr_total, DXM, 512), f16)
    txt[:, :DX, :] = tgtx.reshape(ngr_total, 4, T, DX).transpose(
        0, 3, 1, 2).reshape(ngr_total, DX, 512)
    txt[:, DX:, :] = bm[np.arange(ngr_total) // 4]

    sidx = np.arange(SC) % 16
    a16x = (np.arange(16)[:, None, None] == sidx[None, :, None]).astype(f16)
    a16x = np.ascontiguousarray(np.broadcast_to(a16x, (16, SC, H)))

    ipw = np.asarray(in_proj_weight, f)
    ipb = np.asarray(in_proj_bias, f)
    opw = np.asarray(out_proj_weight, f)
    opb = np.asarray(out_proj_bias, f)
    rw = np.asarray(rpe_weight, f)
    rb = np.asarray(rpe_bias, f)

    wsrcT = np.ascontiguousarray(ipw[:D].T * scale)          # [d, e]
    bsrc = np.ascontiguousarray((ipb[:D] * scale)[:, None])  # [D, 1]
    wk = np.ascontiguousarray(ipw[D:2 * D])                  # [e, d]
    rwk = np.ascontiguousarray(rw[:D])                       # [e, r]
    wvx = np.concatenate([ipw[2 * D:3 * D], rw[D:2 * D]], axis=1)  # [e, 288]
    wfx = np.empty((H, DX, DOUT), f)
    for h in range(H):
        hs = slice(h * 32, (h + 1) * 32)
        wfx[h] = (opw[:, hs] @ wvx[hs, :]).T
    obias = (opb + opw @ (ipb[2 * D:3 * D] + rb[D:2 * D]))[None, :]
    obias = np.ascontiguousarray(np.repeat(obias.astype(f), SC, axis=0))

    wfx = round_f32r(wfx)

    ngr = SC // 4
    in_maps = []
    for c in range(NCORES):
        sl = slice(c * SC, (c + 1) * SC)
        in_maps.append({
            "src": src_f[sl],
            "txn": txn[c * ngr:(c + 1) * ngr],
            "txt": txt[c * ngr:(c + 1) * ngr],
            "a16x": a16x,
            "wsrcT": wsrcT,
            "wk": wk,
            "rwk": rwk,
            "wfx": wfx,
            "bsrc": bsrc,
            "obias": obias,
            "rmask": rowmask[sl],
        })
    return in_maps


def round_f32r(x):
    """Round fp32 array to the fp32r grid (RNE to 11 mantissa bits)."""
    u = np.ascontiguousarray(x, np.float32).view(np.uint32)
    u = (u + 0x7FF + ((u >> 12) & 1)) & 0xFFFFF000
    return u.astype(np.uint32).view(np.float32)


_NC_CACHE = {}


def get_nc(sc=SC):
    if sc not in _NC_CACHE:
        _NC_CACHE[sc] = build(sc)
    return _NC_CACHE[sc]


def run(in_maps, trace=False):
    nc = get_nc(SC)
    return run_bass_kernel_spmd(nc, in_maps, list(range(NCORES)), trace=trace)


def kernel(**inputs):
    in_maps = host_prep(**inputs)
    res = run(in_maps).results
    out = np.concatenate([res[c]["out"] for c in range(NCORES)], axis=0)
    return np.ascontiguousarray(out.reshape(B, S, D))
